# revision 2
# baseline (speedup 1.0000x reference)
"""Trainium2 Bass kernel for nn_Conv2d_22222160789797.

Conv2d: x [32,128,56,56] f32, weight [256,128,3,3] (OIHW), stride 1, pad 1
-> out [32,256,56,56] f32.

Strategy: data-parallel over batch across 8 cores (4 images/core), bf16
operands, 1-D Winograd F(2,3) along W (host-side input/weight transform:
4 positions x 28 tiles per row instead of 3 kw taps x 56 cols), and the
3 kh taps accumulated in PSUM. PE work: 48 matmuls of free-dim 392 per
(img, oc-half) = 384 MMs/core ~ 63.4us of streaming at 2.4GHz.

Key structure (v2, from trace analysis of the v1 kernel at ~100us):

- Input layout is pos-major [img, ic, pos, padded_row, 28] in emission
  order [1,3,2,0], so every matmul's moving operand is one flat
  contiguous [392] slice, and image 0 streams in as 4 contiguous
  pos-slice DMAs (406KB each) -- the first real MM can start ~10.3us
  while the HAM clock-gate is still warming anyway.

- MM emission per (img, half) is (pos, kh)-outer / chunk-inner: the 4
  row-chunks' MMs share one LDWEIGHTS (12 weight loads per img-half
  instead of 48), keeping the PE at its ~166ns/MM streaming rate.

- PSUM: two [128, 1024] f32 tiles ("chunk pairs", 2 banks each) x
  bufs=2 = all 8 banks. Chunk c of a pair accumulates its 3 kh taps
  into the bank at offset c*512. Consumers read the pair through a
  [2, 392]-strided view, halving per-op fixed overhead (FD=784 ops
  instead of 2x FD=392: DVE ~960ns vs 1350ns).

- Output transform per pair, engine-legal (DVE one PSUM operand/op,
  ACT stages via activation-copy, GpSimd SBUF-only), pos order
  [1,3,2,0] so the odd-column path (c3, out1) completes mid-stream and
  only out0 = u + m0 (one DVE op) trails the final MM of an img-half:
    ACT:    c1 = m1, c3 = m3          (PSUM->SBUF stages, f32)
    DVE:    v = c1 - m2, u = c1 + m2
    GpSimd: out1(odd cols)  = v - c3  (bf16 out)
    DVE:    out0(even cols) = u + m0  (bf16 out)

- Output is written bf16 (halves the 12.8MB/core f32 output DMA; the
  host upcasts; bf16 rounding adds ~2e-3 rel err vs the 2e-2 gate).
  Output DMAs alternate between the sync and scalar HWDGE rings; input
  DMAs ride sync (weights + image-0 pos slices) and gpsimd (whole-image
  prefetch of images 1-3).
"""

import numpy as np

import concourse.tile as tile
from concourse import bacc, mybir
from concourse.bass_utils import run_bass_kernel_spmd

N_CORES = 8
B, IC, H, W = 32, 128, 56, 56
OC, KH, KW = 256, 3, 3
BPC = B // N_CORES          # images per core
PH = H + 2                  # padded rows
J = 28                      # winograd tiles per row (2 output cols each)
R = 14                      # output rows per chunk
N_CHUNKS = H // R           # 4
N_PAIRS = N_CHUNKS // 2     # 2 chunk-pairs per (img, half)
OC_HALVES = OC // 128       # 2
FD = R * J                  # 392 matmul free dim
BANK = 512                  # f32 elems per PSUM bank

# winograd position order as laid out on host / emitted on device
POS_SEQ = (1, 3, 2, 0)

_f32 = mybir.dt.float32
_bf16 = mybir.dt.bfloat16

_compiled_nc = None

N_WARMUP = 7


def _build(warmup=N_WARMUP):
    nc = bacc.Bacc("TRN2", target_bir_lowering=False, debug=False)
    # host-transformed input, pos-major: [img, ic, pos(4), padded row, J]
    x_d = nc.dram_tensor("x", [BPC, IC, 4, PH, J], _bf16,
                         kind="ExternalInput")
    # weights: [ic, (half, slice, posinslice, kh, 128)]
    w_d = nc.dram_tensor("w", [IC, OC_HALVES * 2 * 2 * KH * 128], _bf16,
                         kind="ExternalInput")
    o_d = nc.dram_tensor("out", [BPC, OC, H * W], _bf16,
                         kind="ExternalOutput")
    w6 = w_d[:].rearrange("p (h s q k c) -> p h s q k c",
                          h=OC_HALVES, s=2, q=2, k=KH, c=128)

    with tile.TileContext(nc) as tc:
        with (
            tc.tile_pool(name="w", bufs=1) as wpool,
            tc.tile_pool(name="x", bufs=1) as xpool,
            tc.tile_pool(name="o", bufs=4) as opool,
            tc.tile_pool(name="ps", bufs=8, space="PSUM") as pspool,
        ):
            if warmup:
                wscr = wpool.tile([128, 128], _bf16, name="wscr", tag="wscr")
                xscr = wpool.tile([128, FD], _bf16, name="xscr", tag="xscr")
                nc.gpsimd.memset(wscr[:], 0.0)
                nc.gpsimd.memset(xscr[:], 0.0)
                for _ in range(warmup):
                    pwarm = pspool.tile([128, 2 * BANK], _f32, name="pwarm",
                                        tag="pp0", bufs=2)
                    nc.tensor.matmul(pwarm[:, 0:FD], wscr[:], xscr[:],
                                     start=True, stop=True)

            # weight tiles: per (half, slice) = 2 pos blocks x 3 kh x 128 oc
            whAB = []
            for half in range(OC_HALVES):
                a = wpool.tile([IC, 2, KH, 128], _bf16, name=f"wh{half}a",
                               tag=f"wh{half}a")
                bb = wpool.tile([IC, 2, KH, 128], _bf16, name=f"wh{half}b",
                                tag=f"wh{half}b")
                whAB.append((a, bb))

            # whole-image tiles, pos-major; image 0 arrives as 4 pos-slice
            # DMAs on the sync ring interleaved with the weight slices so
            # the first (pos, weight) pair gates on ~600KB; images 1-3
            # prefetch on the gpsimd ring in parallel.
            xt = [
                xpool.tile([IC, 4, PH, J], _bf16, name=f"xt{img}",
                           tag=f"xt{img}")
                for img in range(BPC)
            ]
            nc.sync.dma_start(whAB[0][0][:], w6[:, 0, 0])
            nc.sync.dma_start(xt[0][:, 0], x_d[0, :, 0])
            nc.sync.dma_start(xt[0][:, 1], x_d[0, :, 1])
            nc.sync.dma_start(whAB[0][1][:], w6[:, 0, 1])
            nc.sync.dma_start(xt[0][:, 2], x_d[0, :, 2])
            nc.sync.dma_start(xt[0][:, 3], x_d[0, :, 3])
            nc.sync.dma_start(whAB[1][0][:], w6[:, 1, 0])
            nc.sync.dma_start(whAB[1][1][:], w6[:, 1, 1])
            for img in range(1, BPC):
                nc.gpsimd.dma_start(xt[img][:], x_d[img])

            out_slot = 0

            def img_half(img, half):
                nonlocal out_slot
                # PSUM pair tiles: [128, 2 banks]; chunk ci of the pair
                # accumulates at offset ci*512. Consumers view them as
                # [2, 392] strided.
                pp = {}

                def pos_block(qi):
                    tiles = []
                    for pair in range(N_PAIRS):
                        p = pspool.tile([128, 2 * BANK], _f32,
                                        name=f"pp{pair}", tag=f"pp{pair}",
                                        bufs=2)
                        tiles.append(p)
                    pp[qi] = tiles
                    s, qq = qi // 2, qi % 2
                    for kh in range(KH):
                        wt = whAB[half][s][:, qq, kh, :]
                        for pair in range(N_PAIRS):
                            for ci in range(2):
                                chunk = pair * 2 + ci
                                r0 = chunk * R + kh
                                nc.tensor.matmul(
                                    pp[qi][pair][:, ci * BANK:ci * BANK + FD],
                                    wt,
                                    xt[img][:, qi, r0:r0 + R, :],
                                    start=(kh == 0),
                                    stop=(kh == KH - 1),
                                )

                def pview(qi, pair):
                    return pp[qi][pair].rearrange(
                        "p (b x) -> p b x", b=2, x=BANK)[:, :, 0:FD]

                ots = [opool.tile([128, 2 * R * W], _bf16, name="ot",
                                  tag="ot", bufs=6) for _ in range(N_PAIRS)]

                # pos 1: stage c1
                pos_block(0)
                c1s = []
                for pair in range(N_PAIRS):
                    c1 = opool.tile([128, 2 * FD], _f32, name="c1", tag="c1",
                                    bufs=3)
                    nc.scalar.copy(c1[:], pview(0, pair))
                    c1s.append(c1)
                # pos 3: stage c3
                pos_block(1)
                c3s = []
                for pair in range(N_PAIRS):
                    c3 = opool.tile([128, 2 * FD], _f32, name="c3", tag="c3",
                                    bufs=3)
                    nc.scalar.copy(c3[:], pview(1, pair))
                    c3s.append(c3)
                # pos 2: v, u then odd cols (GpSimd, SBUF-only)
                pos_block(2)
                us = []
                for pair in range(N_PAIRS):
                    v = opool.tile([128, 2 * FD], _f32, name="v", tag="v",
                                   bufs=3)
                    nc.vector.tensor_sub(v[:], c1s[pair][:], pview(2, pair))
                    u = opool.tile([128, 2 * FD], _f32, name="u", tag="u",
                                   bufs=3)
                    nc.vector.tensor_add(u[:], c1s[pair][:], pview(2, pair))
                    us.append(u)
                    nc.gpsimd.tensor_sub(
                        ots[pair][:, 1:2 * R * W:2], v[:], c3s[pair][:])
                # pos 0: even cols, then ship the pair
                pos_block(3)
                for pair in range(N_PAIRS):
                    nc.vector.tensor_add(
                        ots[pair][:, 0:2 * R * W:2], us[pair][:],
                        pview(3, pair))
                    ring = nc.scalar if out_slot % 2 == 0 else nc.sync
                    ring.dma_start(
                        o_d[img, half * 128:half * 128 + 128,
                            pair * 2 * R * W:(pair + 1) * 2 * R * W],
                        ots[pair][:],
                    )
                    out_slot += 1

            for img in range(BPC):
                for half in range(OC_HALVES):
                    img_half(img, half)
    nc.compile()
    return nc


def _get_nc():
    global _compiled_nc
    if _compiled_nc is None:
        _compiled_nc = _build()
    return _compiled_nc


def _prep_inputs(x, weight):
    import ml_dtypes

    x = np.asarray(x, dtype=np.float32)
    weight = np.asarray(weight, dtype=np.float32)
    xp = np.zeros((B, IC, PH, PH), dtype=np.float32)
    xp[:, :, 1:H + 1, 1:W + 1] = x
    # host winograd input transform along W, pos-major in emission order
    # [1, 3, 2, 0]: [B, IC, 4, PH, J]
    d0 = xp[:, :, :, 0:2 * J - 1:2]
    d1 = xp[:, :, :, 1:2 * J:2]
    d2 = xp[:, :, :, 2:2 * J + 1:2]
    d3 = xp[:, :, :, 3:2 * J + 2:2]
    xt = np.empty((B, IC, 4, PH, J), dtype=ml_dtypes.bfloat16)
    xt[:, :, 0] = d1 + d2          # pos 1
    xt[:, :, 1] = d1 - d3          # pos 3
    xt[:, :, 2] = d2 - d1          # pos 2
    xt[:, :, 3] = d0 - d2          # pos 0
    # host winograd weight transform: Wt[ic, kh, pos, oc] = sum_kw G[pos,kw] w
    G = np.array(
        [[1, 0, 0], [0.5, 0.5, 0.5], [0.5, -0.5, 0.5], [0, 0, 1]],
        dtype=np.float32,
    )
    # -> [ic, half, slice, posinslice, kh, 128] with pos order [1, 3, 2, 0]
    wt = (
        np.einsum("pw,oihw->ihpo", G, weight)
        .reshape(IC, KH, 4, OC_HALVES, 128)[:, :, [1, 3, 2, 0]]
        .transpose(0, 3, 2, 1, 4)          # [ic, half, pos^, kh, 128]
        .reshape(IC, OC_HALVES, 2, 2, KH, 128)
        .reshape(IC, OC_HALVES * 2 * 2 * KH * 128)
        .astype(ml_dtypes.bfloat16)
    )
    in_maps = [
        {"x": np.ascontiguousarray(xt[c * BPC:(c + 1) * BPC]), "w": wt}
        for c in range(N_CORES)
    ]
    return in_maps


def _run(x, weight, trace=False):
    nc = _get_nc()
    in_maps = _prep_inputs(x, weight)
    res = run_bass_kernel_spmd(nc, in_maps, list(range(N_CORES)), trace=trace)
    out = np.concatenate(
        [np.asarray(res.results[c]["out"]) for c in range(N_CORES)], axis=0
    ).astype(np.float32).reshape(B, OC, H, W)
    return out, res


def kernel(x, weight):
    out, _ = _run(x, weight)
    return out


# revision 5
# speedup vs baseline: 1.1558x; 1.1558x over previous
"""Trainium2 Bass kernel for nn_Conv2d_22222160789797.

Conv2d: x [32,128,56,56] f32, weight [256,128,3,3] (OIHW), stride 1, pad 1
-> out [32,256,56,56] f32.

Strategy: data-parallel over batch across 8 cores (4 images/core), bf16
operands, 1-D Winograd F(2,3) along W (host-side input/weight transform:
4 positions x 28 tiles per row instead of 3 kw taps x 56 cols), and the
3 kh taps accumulated in PSUM. PE work: 48 matmuls of free-dim 392 per
(img, oc-half) = 384 MMs/core ~ 63.4us of streaming at 2.4GHz.

Key structure (v2, from trace analysis of the v1 kernel at ~100us):

- Input layout is pos-major [img, ic, pos, padded_row, 28] in emission
  order [1,3,2,0], so every matmul's moving operand is one flat
  contiguous [392] slice, and image 0 streams in as 4 contiguous
  pos-slice DMAs (406KB each) -- the first real MM can start ~10.3us
  while the HAM clock-gate is still warming anyway.

- MM emission per (img, half) is (pos, kh)-outer / chunk-inner: the 4
  row-chunks' MMs share one LDWEIGHTS (12 weight loads per img-half
  instead of 48), keeping the PE at its ~166ns/MM streaming rate.

- PSUM: two [128, 1024] f32 tiles ("chunk pairs", 2 banks each) x
  bufs=2 = all 8 banks. Chunk c of a pair accumulates its 3 kh taps
  into the bank at offset c*512. Consumers read the pair through a
  [2, 392]-strided view, halving per-op fixed overhead (FD=784 ops
  instead of 2x FD=392: DVE ~960ns vs 1350ns).

- Output transform per pair, engine-legal (DVE one PSUM operand/op,
  ACT stages via activation-copy, GpSimd SBUF-only), pos order
  [1,3,2,0] so the odd-column path (c3, out1) completes mid-stream and
  only out0 = u + m0 (one DVE op) trails the final MM of an img-half:
    ACT:    c1 = m1, c3 = m3          (PSUM->SBUF stages, f32)
    DVE:    v = c1 - m2, u = c1 + m2
    GpSimd: out1(odd cols)  = v - c3  (bf16 out)
    DVE:    out0(even cols) = u + m0  (bf16 out)

- Output is written bf16 (halves the 12.8MB/core f32 output DMA; the
  host upcasts; bf16 rounding adds ~2e-3 rel err vs the 2e-2 gate).
  Output DMAs alternate between the sync and scalar HWDGE rings; input
  DMAs ride sync (weights + image-0 pos slices) and gpsimd (whole-image
  prefetch of images 1-3).
"""

import numpy as np

import concourse.tile as tile
from concourse import bacc, mybir
from concourse.bass_utils import run_bass_kernel_spmd

N_CORES = 8
B, IC, H, W = 32, 128, 56, 56
OC, KH, KW = 256, 3, 3
BPC = B // N_CORES          # images per core
PH = H + 2                  # padded rows
J = 28                      # winograd tiles per row (2 output cols each)
R = 14                      # output rows per chunk
N_CHUNKS = H // R           # 4
N_PAIRS = N_CHUNKS // 2     # 2 chunk-pairs per (img, half)
OC_HALVES = OC // 128       # 2
FD = R * J                  # 392 matmul free dim
BANK = 512                  # f32 elems per PSUM bank

# winograd position order as laid out on host / emitted on device
POS_SEQ = (1, 3, 2, 0)

_f32 = mybir.dt.float32
_bf16 = mybir.dt.bfloat16

_compiled_nc = None

N_WARMUP = 7


def _build(warmup=N_WARMUP):
    nc = bacc.Bacc("TRN2", target_bir_lowering=False, debug=False)
    # host-transformed input, pos-major: [img, ic, pos(4), padded row, J]
    x_d = nc.dram_tensor("x", [BPC, IC, 4, PH, J], _bf16,
                         kind="ExternalInput")
    # weights: [ic, (half, slice, posinslice, kh, 128)]
    w_d = nc.dram_tensor("w", [IC, OC_HALVES * 2 * 2 * KH * 128], _bf16,
                         kind="ExternalInput")
    o_d = nc.dram_tensor("out", [BPC, OC, H * W], _bf16,
                         kind="ExternalOutput")
    w6 = w_d[:].rearrange("p (h s q k c) -> p h s q k c",
                          h=OC_HALVES, s=2, q=2, k=KH, c=128)

    with tile.TileContext(nc) as tc:
        with (
            tc.tile_pool(name="w", bufs=1) as wpool,
            tc.tile_pool(name="x", bufs=1) as xpool,
            tc.tile_pool(name="o", bufs=4) as opool,
            tc.tile_pool(name="ps", bufs=8, space="PSUM") as pspool,
        ):
            if warmup:
                wscr = wpool.tile([128, 128], _bf16, name="wscr", tag="wscr")
                xscr = wpool.tile([128, FD], _bf16, name="xscr", tag="xscr")
                nc.gpsimd.memset(wscr[:], 0.0)
                nc.gpsimd.memset(xscr[:], 0.0)
                for _ in range(warmup):
                    pwarm = pspool.tile([128, 2 * BANK], _f32, name="pwarm",
                                        tag="pp0", bufs=2)
                    nc.tensor.matmul(pwarm[:, 0:FD], wscr[:], xscr[:],
                                     start=True, stop=True)

            # weight tiles: per (half, slice) = 2 pos blocks x 3 kh x 128 oc
            whAB = []
            for half in range(OC_HALVES):
                a = wpool.tile([IC, 2, KH, 128], _bf16, name=f"wh{half}a",
                               tag=f"wh{half}a")
                bb = wpool.tile([IC, 2, KH, 128], _bf16, name=f"wh{half}b",
                                tag=f"wh{half}b")
                whAB.append((a, bb))

            # whole-image tiles, pos-major. ALL input DMAs ride the sync
            # ring in strict need-order: HWDGE drains one ring FIFO, so
            # queue position IS priority -- the image-1..3 prefetches sit
            # behind the critical image-0 pos slices + weight slices
            # instead of stealing SDMA packets from them (the v2 mistake:
            # xt prefetch on the gpsimd ring round-robined against the
            # head slices at packet granularity and delayed the first real
            # MM by ~10us).
            xt = [
                xpool.tile([IC, 4, PH, J], _bf16, name=f"xt{img}",
                           tag=f"xt{img}")
                for img in range(BPC)
            ]
            nc.sync.dma_start(whAB[0][0][:], w6[:, 0, 0])
            nc.sync.dma_start(xt[0][:, 0], x_d[0, :, 0])
            nc.sync.dma_start(xt[0][:, 1], x_d[0, :, 1])
            nc.sync.dma_start(whAB[0][1][:], w6[:, 0, 1])
            nc.sync.dma_start(xt[0][:, 2], x_d[0, :, 2])
            nc.sync.dma_start(xt[0][:, 3], x_d[0, :, 3])
            nc.sync.dma_start(whAB[1][0][:], w6[:, 1, 0])
            nc.sync.dma_start(whAB[1][1][:], w6[:, 1, 1])
            for img in range(1, BPC):
                nc.sync.dma_start(xt[img][:], x_d[img])

            out_slot = 0

            def img_half(img, half, pairs):
                """Emit the MM blocks + output transform for `pairs` (a
                subset of the chunk pairs) of one (img, oc-half)."""
                nonlocal out_slot
                # PSUM pair tiles: [128, 2 banks]; chunk ci of the pair
                # accumulates at offset ci*512. Consumers view them as
                # [2, 392] strided.
                pp = {}

                def pos_block(qi):
                    pp[qi] = {}
                    for pair in pairs:
                        pp[qi][pair] = pspool.tile(
                            [128, 2 * BANK], _f32, name=f"pp{pair}",
                            tag=f"pp{pair}", bufs=2)
                    s, qq = qi // 2, qi % 2
                    for kh in range(KH):
                        wt = whAB[half][s][:, qq, kh, :]
                        for pair in pairs:
                            for ci in range(2):
                                chunk = pair * 2 + ci
                                r0 = chunk * R + kh
                                nc.tensor.matmul(
                                    pp[qi][pair][:, ci * BANK:ci * BANK + FD],
                                    wt,
                                    xt[img][:, qi, r0:r0 + R, :],
                                    start=(kh == 0),
                                    stop=(kh == KH - 1),
                                )

                def pview(qi, pair):
                    return pp[qi][pair].rearrange(
                        "p (b x) -> p b x", b=2, x=BANK)[:, :, 0:FD]

                ots = {pair: opool.tile([128, 2 * R * W], _bf16, name="ot",
                                        tag="ot", bufs=10)
                       for pair in pairs}

                # pos 1: stage c1
                pos_block(0)
                c1s = {}
                for pair in pairs:
                    c1 = opool.tile([128, 2 * FD], _f32, name="c1", tag="c1",
                                    bufs=4)
                    nc.scalar.copy(c1[:], pview(0, pair))
                    c1s[pair] = c1
                # pos 3: stage c3
                pos_block(1)
                c3s = {}
                for pair in pairs:
                    c3 = opool.tile([128, 2 * FD], _f32, name="c3", tag="c3",
                                    bufs=4)
                    nc.scalar.copy(c3[:], pview(1, pair))
                    c3s[pair] = c3
                # pos 2: v, u then odd cols (GpSimd, SBUF-only)
                pos_block(2)
                us = {}
                for pair in pairs:
                    v = opool.tile([128, 2 * FD], _f32, name="v", tag="v",
                                   bufs=4)
                    nc.vector.tensor_sub(v[:], c1s[pair][:], pview(2, pair))
                    u = opool.tile([128, 2 * FD], _f32, name="u", tag="u",
                                   bufs=4)
                    nc.vector.tensor_add(u[:], c1s[pair][:], pview(2, pair))
                    us[pair] = u
                    nc.gpsimd.tensor_sub(
                        ots[pair][:, 1:2 * R * W:2], v[:], c3s[pair][:])
                # pos 0: even cols, then ship the pair
                pos_block(3)
                for pair in pairs:
                    nc.vector.tensor_add(
                        ots[pair][:, 0:2 * R * W:2], us[pair][:],
                        pview(3, pair))
                    # early slots stay off the sync ring (it is draining
                    # the input stream); late slots alternate; the final
                    # two pair DMAs land on different rings.
                    if out_slot < 6:
                        ring = nc.scalar
                    else:
                        ring = nc.scalar if out_slot % 2 == 0 else nc.sync
                    ring.dma_start(
                        o_d[img, half * 128:half * 128 + 128,
                            pair * 2 * R * W:(pair + 1) * 2 * R * W],
                        ots[pair][:],
                    )
                    out_slot += 1

            units = [(img, half, tuple(range(N_PAIRS)))
                     for img in range(BPC) for half in range(OC_HALVES)]
            # split the final (img, half) into two sequential pair-units:
            # the last unit's consumer chain (v/u/out0 on DVE, out1 on
            # GpSimd) then covers only one pair, so it trails the final
            # matmul by ~1us instead of ~3.5us.
            limg, lhalf, _ = units.pop()
            units += [(limg, lhalf, (0,)), (limg, lhalf, (1,))]
            for img, half, pairs in units:
                img_half(img, half, pairs)
    nc.compile()
    return nc


def _get_nc():
    global _compiled_nc
    if _compiled_nc is None:
        _compiled_nc = _build()
    return _compiled_nc


def _prep_inputs(x, weight):
    import ml_dtypes

    x = np.asarray(x, dtype=np.float32)
    weight = np.asarray(weight, dtype=np.float32)
    xp = np.zeros((B, IC, PH, PH), dtype=np.float32)
    xp[:, :, 1:H + 1, 1:W + 1] = x
    # host winograd input transform along W, pos-major in emission order
    # [1, 3, 2, 0]: [B, IC, 4, PH, J]
    d0 = xp[:, :, :, 0:2 * J - 1:2]
    d1 = xp[:, :, :, 1:2 * J:2]
    d2 = xp[:, :, :, 2:2 * J + 1:2]
    d3 = xp[:, :, :, 3:2 * J + 2:2]
    xt = np.empty((B, IC, 4, PH, J), dtype=ml_dtypes.bfloat16)
    xt[:, :, 0] = d1 + d2          # pos 1
    xt[:, :, 1] = d1 - d3          # pos 3
    xt[:, :, 2] = d2 - d1          # pos 2
    xt[:, :, 3] = d0 - d2          # pos 0
    # host winograd weight transform: Wt[ic, kh, pos, oc] = sum_kw G[pos,kw] w
    G = np.array(
        [[1, 0, 0], [0.5, 0.5, 0.5], [0.5, -0.5, 0.5], [0, 0, 1]],
        dtype=np.float32,
    )
    # -> [ic, half, slice, posinslice, kh, 128] with pos order [1, 3, 2, 0]
    wt = (
        np.einsum("pw,oihw->ihpo", G, weight)
        .reshape(IC, KH, 4, OC_HALVES, 128)[:, :, [1, 3, 2, 0]]
        .transpose(0, 3, 2, 1, 4)          # [ic, half, pos^, kh, 128]
        .reshape(IC, OC_HALVES, 2, 2, KH, 128)
        .reshape(IC, OC_HALVES * 2 * 2 * KH * 128)
        .astype(ml_dtypes.bfloat16)
    )
    in_maps = [
        {"x": np.ascontiguousarray(xt[c * BPC:(c + 1) * BPC]), "w": wt}
        for c in range(N_CORES)
    ]
    return in_maps


def _run(x, weight, trace=False):
    nc = _get_nc()
    in_maps = _prep_inputs(x, weight)
    res = run_bass_kernel_spmd(nc, in_maps, list(range(N_CORES)), trace=trace)
    out = np.concatenate(
        [np.asarray(res.results[c]["out"]) for c in range(N_CORES)], axis=0
    ).astype(np.float32).reshape(B, OC, H, W)
    return out, res


def kernel(x, weight):
    out, _ = _run(x, weight)
    return out


# revision 7
# speedup vs baseline: 1.1840x; 1.0244x over previous
"""Trainium2 Bass kernel for nn_Conv2d_22222160789797.

Conv2d: x [32,128,56,56] f32, weight [256,128,3,3] (OIHW), stride 1, pad 1
-> out [32,256,56,56] f32.

Strategy: data-parallel over batch across 8 cores (4 images/core), bf16
operands, 1-D Winograd F(2,3) along W (host-side input/weight transform:
4 positions x 28 tiles per row instead of 3 kw taps x 56 cols), and the
3 kh taps accumulated in PSUM. PE work: 48 matmuls of free-dim 392 per
(img, oc-half) = 384 MMs/core ~ 63.4us of streaming at 2.4GHz.

Key structure (v2, from trace analysis of the v1 kernel at ~100us):

- Input layout is pos-major [img, ic, pos, padded_row, 28] in emission
  order [1,3,2,0], so every matmul's moving operand is one flat
  contiguous [392] slice, and image 0 streams in as 4 contiguous
  pos-slice DMAs (406KB each) -- the first real MM can start ~10.3us
  while the HAM clock-gate is still warming anyway.

- MM emission per (img, half) is (pos, kh)-outer / chunk-inner: the 4
  row-chunks' MMs share one LDWEIGHTS (12 weight loads per img-half
  instead of 48), keeping the PE at its ~166ns/MM streaming rate.

- PSUM: two [128, 1024] f32 tiles ("chunk pairs", 2 banks each) x
  bufs=2 = all 8 banks. Chunk c of a pair accumulates its 3 kh taps
  into the bank at offset c*512. Consumers read the pair through a
  [2, 392]-strided view, halving per-op fixed overhead (FD=784 ops
  instead of 2x FD=392: DVE ~960ns vs 1350ns).

- Output transform per pair, engine-legal (DVE one PSUM operand/op,
  ACT stages via activation-copy, GpSimd SBUF-only), pos order
  [1,3,2,0] so the odd-column path (c3, out1) completes mid-stream and
  only out0 = u + m0 (one DVE op) trails the final MM of an img-half:
    ACT:    c1 = m1, c3 = m3          (PSUM->SBUF stages, f32)
    DVE:    v = c1 - m2, u = c1 + m2
    GpSimd: out1(odd cols)  = v - c3  (bf16 out)
    DVE:    out0(even cols) = u + m0  (bf16 out)

- Output is written bf16 (halves the 12.8MB/core f32 output DMA; the
  host upcasts; bf16 rounding adds ~2e-3 rel err vs the 2e-2 gate).
  Output DMAs alternate between the sync and scalar HWDGE rings; input
  DMAs ride sync (weights + image-0 pos slices) and gpsimd (whole-image
  prefetch of images 1-3).
"""

import numpy as np

import concourse.tile as tile
from concourse import bacc, mybir
from concourse.bass_utils import run_bass_kernel_spmd

N_CORES = 8
B, IC, H, W = 32, 128, 56, 56
OC, KH, KW = 256, 3, 3
BPC = B // N_CORES          # images per core
PH = H + 2                  # padded rows
J = 28                      # winograd tiles per row (2 output cols each)
R = 14                      # output rows per chunk
N_CHUNKS = H // R           # 4
N_PAIRS = N_CHUNKS // 2     # 2 chunk-pairs per (img, half)
OC_HALVES = OC // 128       # 2
FD = R * J                  # 392 matmul free dim
BANK = 512                  # f32 elems per PSUM bank

# winograd position order as laid out on host / emitted on device
POS_SEQ = (1, 3, 2, 0)

_f32 = mybir.dt.float32
_bf16 = mybir.dt.bfloat16

_compiled_nc = None

N_WARMUP = 13


def _build(warmup=N_WARMUP):
    nc = bacc.Bacc("TRN2", target_bir_lowering=False, debug=False)
    # host-transformed input, pos-major: [img, ic, pos(4), padded row, J]
    x_d = nc.dram_tensor("x", [BPC, IC, 4, PH, J], _bf16,
                         kind="ExternalInput")
    # weights: [ic, (half, slice, posinslice, kh, 128)]
    w_d = nc.dram_tensor("w", [IC, OC_HALVES * 2 * 2 * KH * 128], _bf16,
                         kind="ExternalInput")
    o_d = nc.dram_tensor("out", [BPC, OC, H * W], _bf16,
                         kind="ExternalOutput")
    w6 = w_d[:].rearrange("p (h s q k c) -> p h s q k c",
                          h=OC_HALVES, s=2, q=2, k=KH, c=128)

    with tile.TileContext(nc) as tc:
        with (
            tc.tile_pool(name="w", bufs=1) as wpool,
            tc.tile_pool(name="x", bufs=1) as xpool,
            tc.tile_pool(name="o", bufs=4) as opool,
            tc.tile_pool(name="ps", bufs=8, space="PSUM") as pspool,
        ):
            if warmup:
                wscr = wpool.tile([128, 128], _bf16, name="wscr", tag="wscr")
                xscr = wpool.tile([128, FD], _bf16, name="xscr", tag="xscr")
                nc.gpsimd.memset(wscr[:], 0.0)
                nc.gpsimd.memset(xscr[:], 0.0)
                for _ in range(warmup):
                    pwarm = pspool.tile([128, 2 * BANK], _f32, name="pwarm",
                                        tag="pp0", bufs=2)
                    nc.tensor.matmul(pwarm[:, 0:FD], wscr[:], xscr[:],
                                     start=True, stop=True)

            # weight tiles: per (half, slice) = 2 pos blocks x 3 kh x 128 oc
            whAB = []
            for half in range(OC_HALVES):
                a = wpool.tile([IC, 2, KH, 128], _bf16, name=f"wh{half}a",
                               tag=f"wh{half}a")
                bb = wpool.tile([IC, 2, KH, 128], _bf16, name=f"wh{half}b",
                                tag=f"wh{half}b")
                whAB.append((a, bb))

            # whole-image tiles, pos-major. ALL input DMAs ride the sync
            # ring in strict need-order: HWDGE drains one ring FIFO, so
            # queue position IS priority -- the image-1..3 prefetches sit
            # behind the critical image-0 pos slices + weight slices
            # instead of stealing SDMA packets from them (the v2 mistake:
            # xt prefetch on the gpsimd ring round-robined against the
            # head slices at packet granularity and delayed the first real
            # MM by ~10us).
            xt = [
                xpool.tile([IC, 4, PH, J], _bf16, name=f"xt{img}",
                           tag=f"xt{img}")
                for img in range(BPC)
            ]
            nc.sync.dma_start(whAB[0][0][:], w6[:, 0, 0])
            nc.sync.dma_start(xt[0][:, 0], x_d[0, :, 0])
            nc.sync.dma_start(xt[0][:, 1], x_d[0, :, 1])
            nc.sync.dma_start(whAB[0][1][:], w6[:, 0, 1])
            nc.sync.dma_start(xt[0][:, 2], x_d[0, :, 2])
            nc.sync.dma_start(xt[0][:, 3], x_d[0, :, 3])
            nc.sync.dma_start(whAB[1][0][:], w6[:, 1, 0])
            nc.sync.dma_start(whAB[1][1][:], w6[:, 1, 1])
            for img in range(1, BPC):
                nc.sync.dma_start(xt[img][:], x_d[img])

            out_slot = 0

            def unit(img, half, groups):
                """One scheduling unit of an (img, oc-half): `groups` is a
                list of (psum_tag_idx, chunk_tuple). Each group gets a
                2-bank PSUM tile (chunk ci at offset ci*512) and its own
                consumer chain. Emission is group-outer within each pos
                block so a group's consumers start as soon as its own MMs
                stop -- the next block's first group bridges the last
                group's consumer latency instead of stalling on the PSUM
                WAR."""
                nonlocal out_slot
                pp = {}
                c1s, c3s, us = {}, {}, {}
                ots = {}

                def pview(qi, gi, n):
                    return pp[qi, gi].rearrange(
                        "p (b x) -> p b x", b=2, x=BANK)[:, 0:n, 0:FD]

                def emit_group(qi, gi, tagi, chunks):
                    p = pspool.tile([128, 2 * BANK], _f32, name=f"pp{tagi}",
                                    tag=f"pp{tagi}", bufs=2)
                    pp[qi, gi] = p
                    s, qq = qi // 2, qi % 2
                    for kh in range(KH):
                        wt = whAB[half][s][:, qq, kh, :]
                        for ci, chunk in enumerate(chunks):
                            r0 = chunk * R + kh
                            nc.tensor.matmul(
                                p[:, ci * BANK:ci * BANK + FD],
                                wt,
                                xt[img][:, qi, r0:r0 + R, :],
                                start=(kh == 0),
                                stop=(kh == KH - 1),
                            )

                for qi in range(4):
                    for gi, (tagi, chunks) in enumerate(groups):
                        n = len(chunks)
                        emit_group(qi, gi, tagi, chunks)
                        if qi == 0:      # pos 1 -> stage c1
                            c1 = opool.tile([128, n * FD], _f32, name="c1",
                                            tag="c1", bufs=4)
                            nc.scalar.copy(c1[:], pview(0, gi, n))
                            c1s[gi] = c1
                        elif qi == 1:    # pos 3 -> stage c3
                            c3 = opool.tile([128, n * FD], _f32, name="c3",
                                            tag="c3", bufs=4)
                            nc.scalar.copy(c3[:], pview(1, gi, n))
                            c3s[gi] = c3
                        elif qi == 2:    # pos 2 -> v, u, odd cols
                            ot = opool.tile([128, 2 * n * FD], _bf16,
                                            name="ot", tag="ot", bufs=10)
                            ots[gi] = ot
                            v = opool.tile([128, n * FD], _f32, name="v",
                                           tag="v", bufs=4)
                            nc.vector.tensor_sub(v[:], c1s[gi][:],
                                                 pview(2, gi, n))
                            u = opool.tile([128, n * FD], _f32, name="u",
                                           tag="u", bufs=4)
                            nc.vector.tensor_add(u[:], c1s[gi][:],
                                                 pview(2, gi, n))
                            us[gi] = u
                            nc.gpsimd.tensor_sub(
                                ot[:, 1:2 * n * FD:2], v[:], c3s[gi][:])
                        else:            # pos 0 -> even cols, ship
                            ot = ots[gi]
                            n0 = chunks[0] * 2 * FD
                            nc.vector.tensor_add(
                                ot[:, 0:2 * n * FD:2], us[gi][:],
                                pview(3, gi, n))
                            # early slots stay off the sync ring (it is
                            # draining the input stream); later alternate.
                            if out_slot < 6:
                                ring = nc.scalar
                            else:
                                ring = (nc.scalar if out_slot % 2 == 0
                                        else nc.sync)
                            ring.dma_start(
                                o_d[img, half * 128:half * 128 + 128,
                                    n0:n0 + 2 * n * FD],
                                ot[:],
                            )
                            out_slot += 1

            full = [(0, (0, 1)), (1, (2, 3))]
            units = [(img, half, full)
                     for img in range(BPC) for half in range(OC_HALVES)]
            # taper the end: the final (img, half) runs as a pair-unit
            # plus two single-chunk units so the trailing consumer chain
            # after the very last matmul is one FD=392 DVE op + one
            # ~200KB DMA instead of a full pair's ~3.5us chain.
            limg, lhalf, _ = units.pop()
            units += [
                (limg, lhalf, [(0, (0, 1))]),
                (limg, lhalf, [(1, (2,))]),
                (limg, lhalf, [(1, (3,))]),
            ]
            for img, half, groups in units:
                unit(img, half, groups)
    nc.compile()
    return nc


def _get_nc():
    global _compiled_nc
    if _compiled_nc is None:
        _compiled_nc = _build()
    return _compiled_nc


def _prep_inputs(x, weight):
    import ml_dtypes

    x = np.asarray(x, dtype=np.float32)
    weight = np.asarray(weight, dtype=np.float32)
    xp = np.zeros((B, IC, PH, PH), dtype=np.float32)
    xp[:, :, 1:H + 1, 1:W + 1] = x
    # host winograd input transform along W, pos-major in emission order
    # [1, 3, 2, 0]: [B, IC, 4, PH, J]
    d0 = xp[:, :, :, 0:2 * J - 1:2]
    d1 = xp[:, :, :, 1:2 * J:2]
    d2 = xp[:, :, :, 2:2 * J + 1:2]
    d3 = xp[:, :, :, 3:2 * J + 2:2]
    xt = np.empty((B, IC, 4, PH, J), dtype=ml_dtypes.bfloat16)
    xt[:, :, 0] = d1 + d2          # pos 1
    xt[:, :, 1] = d1 - d3          # pos 3
    xt[:, :, 2] = d2 - d1          # pos 2
    xt[:, :, 3] = d0 - d2          # pos 0
    # host winograd weight transform: Wt[ic, kh, pos, oc] = sum_kw G[pos,kw] w
    G = np.array(
        [[1, 0, 0], [0.5, 0.5, 0.5], [0.5, -0.5, 0.5], [0, 0, 1]],
        dtype=np.float32,
    )
    # -> [ic, half, slice, posinslice, kh, 128] with pos order [1, 3, 2, 0]
    wt = (
        np.einsum("pw,oihw->ihpo", G, weight)
        .reshape(IC, KH, 4, OC_HALVES, 128)[:, :, [1, 3, 2, 0]]
        .transpose(0, 3, 2, 1, 4)          # [ic, half, pos^, kh, 128]
        .reshape(IC, OC_HALVES, 2, 2, KH, 128)
        .reshape(IC, OC_HALVES * 2 * 2 * KH * 128)
        .astype(ml_dtypes.bfloat16)
    )
    in_maps = [
        {"x": np.ascontiguousarray(xt[c * BPC:(c + 1) * BPC]), "w": wt}
        for c in range(N_CORES)
    ]
    return in_maps


def _run(x, weight, trace=False):
    nc = _get_nc()
    in_maps = _prep_inputs(x, weight)
    res = run_bass_kernel_spmd(nc, in_maps, list(range(N_CORES)), trace=trace)
    out = np.concatenate(
        [np.asarray(res.results[c]["out"]) for c in range(N_CORES)], axis=0
    ).astype(np.float32).reshape(B, OC, H, W)
    return out, res


def kernel(x, weight):
    out, _ = _run(x, weight)
    return out


# revision 8
# speedup vs baseline: 1.1953x; 1.0095x over previous
"""Trainium2 Bass kernel for nn_Conv2d_22222160789797.

Conv2d: x [32,128,56,56] f32, weight [256,128,3,3] (OIHW), stride 1, pad 1
-> out [32,256,56,56] f32.

Strategy: data-parallel over batch across 8 cores (4 images/core), bf16
operands, 1-D Winograd F(2,3) along W (host-side input/weight transform:
4 positions x 28 tiles per row instead of 3 kw taps x 56 cols), and the
3 kh taps accumulated in PSUM. PE work: 48 matmuls of free-dim 392 per
(img, oc-half) = 384 MMs/core ~ 63.4us of streaming at 2.4GHz.

Key structure (v2, from trace analysis of the v1 kernel at ~100us):

- Input layout is pos-major [img, ic, pos, padded_row, 28] in emission
  order [1,3,2,0], so every matmul's moving operand is one flat
  contiguous [392] slice, and image 0 streams in as 4 contiguous
  pos-slice DMAs (406KB each) -- the first real MM can start ~10.3us
  while the HAM clock-gate is still warming anyway.

- MM emission per (img, half) is (pos, kh)-outer / chunk-inner: the 4
  row-chunks' MMs share one LDWEIGHTS (12 weight loads per img-half
  instead of 48), keeping the PE at its ~166ns/MM streaming rate.

- PSUM: two [128, 1024] f32 tiles ("chunk pairs", 2 banks each) x
  bufs=2 = all 8 banks. Chunk c of a pair accumulates its 3 kh taps
  into the bank at offset c*512. Consumers read the pair through a
  [2, 392]-strided view, halving per-op fixed overhead (FD=784 ops
  instead of 2x FD=392: DVE ~960ns vs 1350ns).

- Output transform per pair, engine-legal (DVE one PSUM operand/op,
  ACT stages via activation-copy, GpSimd SBUF-only), pos order
  [1,3,2,0] so the odd-column path (c3, out1) completes mid-stream and
  only out0 = u + m0 (one DVE op) trails the final MM of an img-half:
    ACT:    c1 = m1, c3 = m3          (PSUM->SBUF stages, f32)
    DVE:    v = c1 - m2, u = c1 + m2
    GpSimd: out1(odd cols)  = v - c3  (bf16 out)
    DVE:    out0(even cols) = u + m0  (bf16 out)

- Output is written bf16 (halves the 12.8MB/core f32 output DMA; the
  host upcasts; bf16 rounding adds ~2e-3 rel err vs the 2e-2 gate).
  Output DMAs alternate between the sync and scalar HWDGE rings; input
  DMAs ride sync (weights + image-0 pos slices) and gpsimd (whole-image
  prefetch of images 1-3).
"""

import numpy as np

import concourse.tile as tile
from concourse import bacc, mybir
from concourse.bass_utils import run_bass_kernel_spmd

N_CORES = 8
B, IC, H, W = 32, 128, 56, 56
OC, KH, KW = 256, 3, 3
BPC = B // N_CORES          # images per core
PH = H + 2                  # padded rows
J = 28                      # winograd tiles per row (2 output cols each)
R = 14                      # output rows per chunk
N_CHUNKS = H // R           # 4
N_PAIRS = N_CHUNKS // 2     # 2 chunk-pairs per (img, half)
OC_HALVES = OC // 128       # 2
FD = R * J                  # 392 matmul free dim
BANK = 512                  # f32 elems per PSUM bank

# winograd position order as laid out on host / emitted on device
POS_SEQ = (1, 3, 2, 0)

_f32 = mybir.dt.float32
_bf16 = mybir.dt.bfloat16

_compiled_nc = None

N_WARMUP = 13


def _build(warmup=N_WARMUP):
    nc = bacc.Bacc("TRN2", target_bir_lowering=False, debug=False)
    # host-transformed input, pos-major: [img, ic, pos(4), padded row, J]
    x_d = nc.dram_tensor("x", [BPC, IC, 4, PH, J], _bf16,
                         kind="ExternalInput")
    # weights: [ic, (half, slice, posinslice, kh, 128)]
    w_d = nc.dram_tensor("w", [IC, OC_HALVES * 2 * 2 * KH * 128], _bf16,
                         kind="ExternalInput")
    o_d = nc.dram_tensor("out", [BPC, OC, H * W], _bf16,
                         kind="ExternalOutput")
    w6 = w_d[:].rearrange("p (h s q k c) -> p h s q k c",
                          h=OC_HALVES, s=2, q=2, k=KH, c=128)

    with tile.TileContext(nc) as tc:
        with (
            tc.tile_pool(name="w", bufs=1) as wpool,
            tc.tile_pool(name="x", bufs=1) as xpool,
            tc.tile_pool(name="o", bufs=4) as opool,
            tc.tile_pool(name="ps", bufs=8, space="PSUM") as pspool,
        ):
            if warmup:
                wscr = wpool.tile([128, 128], _bf16, name="wscr", tag="wscr")
                xscr = wpool.tile([128, FD], _bf16, name="xscr", tag="xscr")
                nc.gpsimd.memset(wscr[:], 0.0)
                nc.gpsimd.memset(xscr[:], 0.0)
                for _ in range(warmup):
                    pwarm = pspool.tile([128, 2 * BANK], _f32, name="pwarm",
                                        tag="pp0", bufs=2)
                    nc.tensor.matmul(pwarm[:, 0:FD], wscr[:], xscr[:],
                                     start=True, stop=True)

            # weight tiles: per (half, slice) = 2 pos blocks x 3 kh x 128 oc
            whAB = []
            for half in range(OC_HALVES):
                a = wpool.tile([IC, 2, KH, 128], _bf16, name=f"wh{half}a",
                               tag=f"wh{half}a")
                bb = wpool.tile([IC, 2, KH, 128], _bf16, name=f"wh{half}b",
                                tag=f"wh{half}b")
                whAB.append((a, bb))

            # whole-image tiles, pos-major. ALL input DMAs ride the sync
            # ring in strict need-order: HWDGE drains one ring FIFO, so
            # queue position IS priority -- the image-1..3 prefetches sit
            # behind the critical image-0 pos slices + weight slices
            # instead of stealing SDMA packets from them (the v2 mistake:
            # xt prefetch on the gpsimd ring round-robined against the
            # head slices at packet granularity and delayed the first real
            # MM by ~10us).
            xt = [
                xpool.tile([IC, 4, PH, J], _bf16, name=f"xt{img}",
                           tag=f"xt{img}")
                for img in range(BPC)
            ]
            nc.sync.dma_start(whAB[0][0][:], w6[:, 0, 0])
            nc.sync.dma_start(xt[0][:, 0], x_d[0, :, 0])
            nc.sync.dma_start(xt[0][:, 1], x_d[0, :, 1])
            nc.sync.dma_start(whAB[0][1][:], w6[:, 0, 1])
            nc.sync.dma_start(xt[0][:, 2], x_d[0, :, 2])
            nc.sync.dma_start(xt[0][:, 3], x_d[0, :, 3])
            nc.sync.dma_start(whAB[1][0][:], w6[:, 1, 0])
            nc.sync.dma_start(whAB[1][1][:], w6[:, 1, 1])
            for img in range(1, BPC):
                nc.sync.dma_start(xt[img][:], x_d[img])

            out_slot = 0

            def unit(img, half, pair, tagi, taper=False):
                """One scheduling unit: a single chunk-pair of an
                (img, oc-half). 4 pos blocks x 6 MMs (kh-outer, so each
                LDWEIGHTS covers 2 MMs), one 2-bank PSUM tile per block.
                Units alternate PSUM tags, so a tag's WAR dependency
                reaches back one full ~4us unit: the previous unit's late
                consumers (v/u/out0 on DVE) never stall this unit's MMs.
                Within the unit, each block's staging copy (c1/c3, ~0.9us
                on ACT) hides under the next ~1us pos block.

                `taper` (final unit): even-column combine + output DMA go
                per-chunk so only one FD=392 DVE op and a ~200KB DMA
                trail the very last matmul."""
                nonlocal out_slot
                pp = {}
                chunks = (2 * pair, 2 * pair + 1)

                def pview(qi, n=2, lo=0):
                    return pp[qi].rearrange(
                        "p (b x) -> p b x", b=2, x=BANK)[:, lo:lo + n, 0:FD]

                for qi in range(4):
                    p = pspool.tile([128, 2 * BANK], _f32, name=f"pp{tagi}",
                                    tag=f"pp{tagi}", bufs=2)
                    pp[qi] = p
                    s, qq = qi // 2, qi % 2
                    for kh in range(KH):
                        wt = whAB[half][s][:, qq, kh, :]
                        for ci, chunk in enumerate(chunks):
                            r0 = chunk * R + kh
                            nc.tensor.matmul(
                                p[:, ci * BANK:ci * BANK + FD],
                                wt,
                                xt[img][:, qi, r0:r0 + R, :],
                                start=(kh == 0),
                                stop=(kh == KH - 1),
                            )
                    if qi == 0:          # pos 1 -> stage c1
                        c1 = opool.tile([128, 2 * FD], _f32, name="c1",
                                        tag="c1", bufs=4)
                        nc.scalar.copy(c1[:], pview(0))
                    elif qi == 1:        # pos 3 -> stage c3
                        c3 = opool.tile([128, 2 * FD], _f32, name="c3",
                                        tag="c3", bufs=4)
                        nc.scalar.copy(c3[:], pview(1))
                    elif qi == 2:        # pos 2 -> v, u, odd cols
                        ot = opool.tile([128, 4 * FD], _bf16, name="ot",
                                        tag="ot", bufs=10)
                        v = opool.tile([128, 2 * FD], _f32, name="v",
                                       tag="v", bufs=4)
                        nc.vector.tensor_sub(v[:], c1[:], pview(2))
                        u = opool.tile([128, 2 * FD], _f32, name="u",
                                       tag="u", bufs=4)
                        nc.vector.tensor_add(u[:], c1[:], pview(2))
                        nc.gpsimd.tensor_sub(ot[:, 1:4 * FD:2], v[:], c3[:])
                    else:                # pos 0 -> even cols, ship
                        osl = o_d[img, half * 128:half * 128 + 128, :]
                        if not taper:
                            nc.vector.tensor_add(ot[:, 0:4 * FD:2], u[:],
                                                 pview(3))
                            if out_slot < 6:
                                ring = nc.scalar
                            else:
                                ring = (nc.scalar if out_slot % 2 == 0
                                        else nc.sync)
                            n0 = chunks[0] * 2 * FD
                            ring.dma_start(osl[:, n0:n0 + 4 * FD], ot[:])
                            out_slot += 1
                        else:
                            for ci, chunk in enumerate(chunks):
                                nc.vector.tensor_add(
                                    ot[:, 2 * ci * FD:2 * (ci + 1) * FD:2],
                                    u[:, ci * FD:(ci + 1) * FD],
                                    pview(3, n=1, lo=ci))
                                ring = nc.scalar if ci == 0 else nc.sync
                                n0 = chunk * 2 * FD
                                ring.dma_start(
                                    osl[:, n0:n0 + 2 * FD],
                                    ot[:, 2 * ci * FD:2 * (ci + 1) * FD])

            uidx = 0
            for img in range(BPC):
                for half in range(OC_HALVES):
                    for pair in range(N_PAIRS):
                        last = (img == BPC - 1 and half == OC_HALVES - 1
                                and pair == N_PAIRS - 1)
                        unit(img, half, pair, uidx % 2, taper=last)
                        uidx += 1
    nc.compile()
    return nc


def _get_nc():
    global _compiled_nc
    if _compiled_nc is None:
        _compiled_nc = _build()
    return _compiled_nc


def _prep_inputs(x, weight):
    import ml_dtypes

    x = np.asarray(x, dtype=np.float32)
    weight = np.asarray(weight, dtype=np.float32)
    xp = np.zeros((B, IC, PH, PH), dtype=np.float32)
    xp[:, :, 1:H + 1, 1:W + 1] = x
    # host winograd input transform along W, pos-major in emission order
    # [1, 3, 2, 0]: [B, IC, 4, PH, J]
    d0 = xp[:, :, :, 0:2 * J - 1:2]
    d1 = xp[:, :, :, 1:2 * J:2]
    d2 = xp[:, :, :, 2:2 * J + 1:2]
    d3 = xp[:, :, :, 3:2 * J + 2:2]
    xt = np.empty((B, IC, 4, PH, J), dtype=ml_dtypes.bfloat16)
    xt[:, :, 0] = d1 + d2          # pos 1
    xt[:, :, 1] = d1 - d3          # pos 3
    xt[:, :, 2] = d2 - d1          # pos 2
    xt[:, :, 3] = d0 - d2          # pos 0
    # host winograd weight transform: Wt[ic, kh, pos, oc] = sum_kw G[pos,kw] w
    G = np.array(
        [[1, 0, 0], [0.5, 0.5, 0.5], [0.5, -0.5, 0.5], [0, 0, 1]],
        dtype=np.float32,
    )
    # -> [ic, half, slice, posinslice, kh, 128] with pos order [1, 3, 2, 0]
    wt = (
        np.einsum("pw,oihw->ihpo", G, weight)
        .reshape(IC, KH, 4, OC_HALVES, 128)[:, :, [1, 3, 2, 0]]
        .transpose(0, 3, 2, 1, 4)          # [ic, half, pos^, kh, 128]
        .reshape(IC, OC_HALVES, 2, 2, KH, 128)
        .reshape(IC, OC_HALVES * 2 * 2 * KH * 128)
        .astype(ml_dtypes.bfloat16)
    )
    in_maps = [
        {"x": np.ascontiguousarray(xt[c * BPC:(c + 1) * BPC]), "w": wt}
        for c in range(N_CORES)
    ]
    return in_maps


def _run(x, weight, trace=False):
    nc = _get_nc()
    in_maps = _prep_inputs(x, weight)
    res = run_bass_kernel_spmd(nc, in_maps, list(range(N_CORES)), trace=trace)
    out = np.concatenate(
        [np.asarray(res.results[c]["out"]) for c in range(N_CORES)], axis=0
    ).astype(np.float32).reshape(B, OC, H, W)
    return out, res


def kernel(x, weight):
    out, _ = _run(x, weight)
    return out
